# revision 11
# baseline (speedup 1.0000x reference)
"""Trainium2 Bass kernel for nn_AdditiveLowRankRoute.

Math: out[b,s,t] = sum_w w_int[w]*silu(ps[b,s,w]*pt[b,t,w]) + s_lin[b,s] + t_lin[b,t] + bias
where ps = source_val @ Ws.T, pt = target_val @ Wt.T,
      s_lin = ps @ ws_out, t_lin = pt @ wt_out.

Key idea: silu(x) = x/2 + r(x) with r even. Fit per-w even polynomials
r(x) ~= sum_m c_{w,m} (x/X_w)^(2m) (coefficient-magnitude-constrained minimax
fits computed on host at call time from the actual data ranges). Then

  sum_w w_int*silu(ps*pt) = sum_w (w_int*ps/2)*pt                 <- 1 matmul
                          + sum_m sum_w [w_int*c_wm*an^2m]*[bn^2m] <- M matmuls

where an = ps/alpha_w, bn = pt/beta_w are computed on device via pre-scaled
projection weights. The whole interaction collapses into a K=(M+1)*128
fp32 matmul accumulated in PSUM; s_lin/t_lin/bias are fused into the PSUM
eviction. Work is sharded across 8 NeuronCores by the source row dim S.
"""
import os
import numpy as np

B, S, T, D, W = 2, 4096, 4096, 512, 128
N_CORES = 8
S_LOC = S // N_CORES          # 512 source rows per core (per batch)
M_POLY = 9                    # even powers 1..M_POLY
KMAX = 600.0                  # L1 coefficient budget per w
MARG = 1.02                   # range margin
OCT = 512                     # t-tile width processed per inner block
N_OCT = T // OCT              # 8
N_SC = S_LOC // 128           # 4 source chunks of 128 rows
N_DC = D // 128               # 4 contraction chunks for projections


def _silu(x):
    return x / (1.0 + np.exp(-x))


def _fit_even_poly(X, M, kmax):
    """Minimax-ish fit of r(x)=silu(x)-x/2 by sum_m c_m (x/X)^(2m) on [-X, X]
    subject to sum|c_m| <= kmax. Returns c[M+1] (m=0..M)."""
    npts = 801
    u = np.cos(np.linspace(0, np.pi, npts))
    r = _silu(u * X) - u * X / 2
    V = np.stack([u ** (2 * m) for m in range(M + 1)], axis=1)
    try:
        from scipy.optimize import linprog

        n = M + 1
        A_ub = np.block([
            [V, -V, -np.ones((npts, 1))],
            [-V, V, -np.ones((npts, 1))],
            [np.ones((1, n)), np.ones((1, n)), np.zeros((1, 1))],
        ])
        b_ub = np.concatenate([r, -r, [kmax]])
        cvec = np.zeros(2 * n + 1)
        cvec[-1] = 1.0
        res = linprog(cvec, A_ub=A_ub, b_ub=b_ub,
                      bounds=[(0, None)] * (2 * n + 1), method="highs")
        if res.status == 0:
            return res.x[:n] - res.x[n:2 * n]
    except Exception:
        pass
    # numpy fallback: IRLS toward minimax + ridge scan for the kappa budget
    best = None
    for lam in np.logspace(-14, -2, 13):
        wts = np.ones(npts)
        c = None
        for _ in range(25):
            A = V * wts[:, None]
            G = A.T @ A + lam * np.eye(M + 1)
            c = np.linalg.solve(G, A.T @ (r * wts))
            res_ = np.abs(V @ c - r)
            wts = np.sqrt(wts * np.maximum(res_, 1e-12)
                          / np.maximum(res_.mean(), 1e-12))
            wts /= wts.mean()
        k = np.abs(c).sum()
        err = np.abs(V @ c - r).max()
        if k <= kmax and (best is None or err < best[1]):
            best = (c, err)
    assert best is not None
    return best[0]


# ----------------------------------------------------------------------------
# Device program
# ----------------------------------------------------------------------------
_PROG_CACHE = {}


def _build_program():
    import concourse.bacc as bacc
    import concourse.mybir as mybir
    import concourse.tile as tile

    fp32 = mybir.dt.float32
    AF = mybir.ActivationFunctionType
    ALU = mybir.AluOpType

    QT = 1024                  # t width per quarter (tgt load + out flush unit)
    N_Q = T // QT              # 4
    OPQ = QT // OCT            # octs per quarter: 2

    nc = bacc.Bacc(None, target_bir_lowering=False)
    reps = int(os.environ.get("ROUTE_REPS", "1"))
    mode = os.environ.get("ROUTE_MODE", "fp32")
    feat_dt = mybir.dt.float32r if mode == "f32r" else fp32
    salt = os.environ.get("ROUTE_BUILD_SALT", "")
    salt_d = None
    if salt:
        salt_d = nc.dram_tensor(f"salt_{salt}", (128, 1), fp32,
                                kind="ExternalInput")
    srcT_d = nc.dram_tensor("srcT", (B, N_DC, 128, S_LOC), fp32, kind="ExternalInput")
    tgtT_d = nc.dram_tensor("tgtT", (B, N_DC, 128, T), fp32, kind="ExternalInput")
    wsnT_d = nc.dram_tensor("wsnT", (N_DC, 128, W), fp32, kind="ExternalInput")
    wtnT_d = nc.dram_tensor("wtnT", (N_DC, 128, W), fp32, kind="ExternalInput")
    # per-partition (w) columns: 0=linA, 1=mpt, 2=wso_mv, 3..3+M-1=coefA(m=1..M),
    # 15=affine const (replicated)
    cols_d = nc.dram_tensor("cols", (W, 16), fp32, kind="ExternalInput")
    wtoR_d = nc.dram_tensor("wtoRep", (W, 128), fp32, kind="ExternalInput")
    out_d = nc.dram_tensor("out", (B, S_LOC, T), fp32, kind="ExternalOutput")

    with tile.TileContext(nc) as tc:
        with (
            tc.tile_pool(name="const", bufs=1) as cpool,
            tc.tile_pool(name="aside", bufs=1) as apool,
            tc.tile_pool(name="achain", bufs=2) as acpool,
            tc.tile_pool(name="bside", bufs=int(os.environ.get("ROUTE_BBUFS", "2")) ) as bpool,
            tc.tile_pool(name="tgtp", bufs=2) as tpool,
            tc.tile_pool(name="srcp", bufs=1) as spool,
            tc.tile_pool(name="stgp", bufs=1) as gpool,
            tc.tile_pool(name="ps_big", bufs=int(os.environ.get("ROUTE_PSBIG", "3")), space="PSUM") as ps_big,
            tc.tile_pool(name="ps_proj", bufs=2, space="PSUM") as ps_proj,
            tc.tile_pool(name="ps_tb", bufs=1, space="PSUM") as ps_tb,
            tc.tile_pool(name="ps_sl", bufs=1, space="PSUM") as ps_sl,
        ):
            wsnT = cpool.tile([128, N_DC, W], fp32, tag="wsnT")
            wtnT = cpool.tile([128, N_DC, W], fp32, tag="wtnT")
            cols = cpool.tile([W, 16], fp32, tag="cols")
            wtoR = cpool.tile([W, 128], fp32, tag="wtoR")
            for c in range(N_DC):
                nc.sync.dma_start(wsnT[:, c, :], wsnT_d[c])
                nc.sync.dma_start(wtnT[:, c, :], wtnT_d[c])
            nc.sync.dma_start(cols[:], cols_d[:])
            nc.sync.dma_start(wtoR[:], wtoR_d[:])
            if salt_d is not None:
                salt_t = cpool.tile([128, 1], fp32, tag="salt")
                nc.sync.dma_start(salt_t[:], salt_d[:])

            for _rep in range(reps):
                for b in range(B):
                    # ---- A side: an[w, s] for this b ----
                    srcT = spool.tile([128, N_DC, S_LOC], fp32, tag="srcT")
                    for c in range(N_DC):
                        nc.sync.dma_start(srcT[:, c, :], srcT_d[b, c])
                    pa_n = ps_proj.tile([128, S_LOC], fp32, tag="p_proj")
                    for c in range(N_DC):
                        nc.tensor.matmul(pa_n[:], wsnT[:, c, :], srcT[:, c, :],
                                         start=(c == 0), stop=(c == N_DC - 1))
                    an = apool.tile([W, S_LOC], fp32, tag="an")
                    nc.scalar.copy(an[:], pa_n[:])

                    # s_lin columns, one per source chunk: [128, 1] each
                    slin = apool.tile([W, N_SC], fp32, tag="slin")
                    for sc in range(N_SC):
                        p_sl = ps_sl.tile([128, 1], fp32, tag="p_sl")
                        nc.tensor.matmul(p_sl[:], an[:, sc * 128:(sc + 1) * 128],
                                         cols[:, 2:3], start=True, stop=True)
                        nc.scalar.copy(slin[:, sc:sc + 1], p_sl[:])

                    # A features: Af0 = linA*an ; Af[m] = coefA_m * (an^2)^m
                    a2 = apool.tile([W, S_LOC], fp32, tag="a2")
                    nc.vector.tensor_mul(a2[:], an[:], an[:])
                    afs = []
                    af0 = apool.tile([W, S_LOC], feat_dt, tag="af0")
                    nc.vector.tensor_scalar_mul(af0[:], an[:], cols[:, 0:1])
                    afs.append(af0)
                    pa_prev = a2
                    for m in range(1, M_POLY + 1):
                        if m > 1:
                            pa_m = acpool.tile([W, S_LOC], fp32, tag="pachain")
                            nc.vector.tensor_mul(pa_m[:], pa_prev[:], a2[:])
                            pa_prev = pa_m
                        af = apool.tile([W, S_LOC], feat_dt, tag=f"af{m}")
                        nc.vector.tensor_scalar_mul(af[:], pa_prev[:],
                                                    cols[:, 2 + m:3 + m])
                        afs.append(af)

                    # ---- B side + big matmul, per t quarter ----
                    for q in range(N_Q):
                        tq0 = q * QT
                        tgtT = tpool.tile([128, N_DC, QT], fp32, tag="tgtT")
                        for c in range(N_DC):
                            nc.sync.dma_start(tgtT[:, c, :],
                                              tgtT_d[b, c, :, tq0:tq0 + QT])
                        stgs = [gpool.tile([128, QT], fp32, tag=f"stg{sc}",
                                           name=f"stg{b}_{q}_{sc}")
                                for sc in range(N_SC)]
                        for o in range(OPQ):
                            t0 = o * OCT
                            p_bn = ps_proj.tile([128, OCT], fp32, tag="p_proj")
                            for c in range(N_DC):
                                nc.tensor.matmul(p_bn[:],
                                                 wtnT[:, c, :],
                                                 tgtT[:, c, t0:t0 + OCT],
                                                 start=(c == 0), stop=(c == N_DC - 1))
                            bn = bpool.tile([W, OCT], fp32, tag="bn")
                            nc.scalar.copy(bn[:], p_bn[:])

                            # tbase[j, t] = t_lin[t] (all rows equal) + (bias+const)
                            p_tb = ps_tb.tile([128, OCT], fp32, tag="p_tb")
                            nc.tensor.matmul(p_tb[:], wtoR[:], bn[:],
                                             start=True, stop=True)
                            tbase = bpool.tile([128, OCT], fp32, tag="tbase")
                            nc.scalar.activation(tbase[:], p_tb[:], AF.Identity,
                                                 bias=cols[:, 15:16])

                            blin = bpool.tile([W, OCT], feat_dt, tag="blin")
                            nc.vector.tensor_scalar_mul(blin[:], bn[:], cols[:, 1:2])
                            # square-tree: fp32 powers of b2 at {1,2,3,4,8} via
                            # ACT Square + DVE muls; features composed with a
                            # single rounding into feat_dt
                            p = {}
                            for mm_ in (1, 2, 4, 8):
                                p[mm_] = bpool.tile([W, OCT], fp32, tag=f"p{mm_}",
                                                    name=f"p{mm_}_{b}_{q}_{o}")
                            nc.scalar.square(p[1][:], bn[:])
                            nc.scalar.square(p[2][:], p[1][:])
                            nc.scalar.square(p[4][:], p[2][:])
                            nc.scalar.square(p[8][:], p[4][:])
                            p[3] = bpool.tile([W, OCT], fp32, tag="p3",
                                              name=f"p3_{b}_{q}_{o}")
                            nc.vector.tensor_mul(p[3][:], p[1][:], p[2][:])
                            comp = {5: (1, 4), 6: (2, 4), 7: (3, 4), 9: (1, 8),
                                    10: (2, 8), 11: (3, 8), 12: (4, 8)}
                            bfs = [blin]
                            for m in range(1, M_POLY + 1):
                                if m in p:
                                    if feat_dt is fp32:
                                        bf = p[m]
                                    else:
                                        bf = bpool.tile([W, OCT], feat_dt,
                                                        tag=f"bf{m}",
                                                        name=f"bf{m}_{b}_{q}_{o}")
                                        nc.vector.tensor_copy(bf[:], p[m][:])
                                else:
                                    i, j = comp[m]
                                    bf = bpool.tile([W, OCT], feat_dt,
                                                    tag=f"bf{m}",
                                                    name=f"bf{m}_{b}_{q}_{o}")
                                    nc.vector.tensor_mul(bf[:], p[i][:], p[j][:])
                                bfs.append(bf)

                            for sc in range(N_SC):
                                po = ps_big.tile([128, OCT], fp32, tag="po")
                                s_sl = slice(sc * 128, (sc + 1) * 128)
                                nc.tensor.matmul(po[:], afs[0][:, s_sl], blin[:],
                                                 start=True, stop=False)
                                for m in range(1, M_POLY + 1):
                                    nc.tensor.matmul(po[:], afs[m][:, s_sl],
                                                     bfs[m][:],
                                                     start=False, stop=(m == M_POLY))
                                nc.vector.scalar_tensor_tensor(
                                    stgs[sc][:, t0:t0 + OCT], po[:],
                                    slin[:, sc:sc + 1], tbase[:],
                                    op0=ALU.add, op1=ALU.add)
                        for sc in range(N_SC):
                            nc.scalar.dma_start(
                                out_d[b, sc * 128:(sc + 1) * 128, tq0:tq0 + QT],
                                stgs[sc][:])

    nc.compile()
    return nc


def _prep_constants(source_val, target_val, Ws, Wt, ws_out, wt_out, w_int, bias):
    """Host-side: data ranges, polynomial fits, packed constant tensors."""
    ps = np.einsum("bsd,wd->bsw", source_val, Ws).astype(np.float32)
    pt = np.einsum("btd,wd->btw", target_val, Wt).astype(np.float32)
    mps = np.abs(ps).max(axis=(0, 1)).astype(np.float64) * MARG
    mpt = np.abs(pt).max(axis=(0, 1)).astype(np.float64) * MARG
    mps = np.maximum(mps, 1e-6)
    mpt = np.maximum(mpt, 1e-6)
    Xw = mps * mpt

    CO = np.zeros((W, M_POLY + 1))
    for w in range(W):
        CO[w] = _fit_even_poly(Xw[w], M_POLY, KMAX)

    w_int64 = w_int.astype(np.float64)
    cols = np.zeros((W, 16), np.float64)
    cols[:, 0] = w_int64 * mps / 2.0                      # linA (scales an -> A_lin)
    cols[:, 1] = mpt                                      # bn -> pt
    cols[:, 2] = mps * ws_out.astype(np.float64)          # s_lin moving vector
    for m in range(1, M_POLY + 1):
        cols[:, 2 + m] = w_int64 * CO[:, m]               # coefA m=1..M
    const_term = float((w_int64 * CO[:, 0]).sum() + float(bias))
    cols[:, 15] = const_term
    wtoRep = np.repeat((mpt * wt_out.astype(np.float64))[:, None], 128, axis=1)

    wsnT = np.ascontiguousarray(
        (Ws.astype(np.float64) / mps[:, None]).T.reshape(N_DC, 128, W))
    wtnT = np.ascontiguousarray(
        (Wt.astype(np.float64) / mpt[:, None]).T.reshape(N_DC, 128, W))
    return (cols.astype(np.float32), wtoRep.astype(np.float32),
            wsnT.astype(np.float32), wtnT.astype(np.float32))


def prepare(source_val, target_val, Ws, Wt, ws_out, wt_out, w_int, bias):
    source_val = np.ascontiguousarray(np.asarray(source_val, np.float32))
    target_val = np.ascontiguousarray(np.asarray(target_val, np.float32))
    Ws = np.asarray(Ws, np.float32)
    Wt = np.asarray(Wt, np.float32)
    ws_out = np.asarray(ws_out, np.float32)
    wt_out = np.asarray(wt_out, np.float32)
    w_int = np.asarray(w_int, np.float32)

    cols, wtoRep, wsnT, wtnT = _prep_constants(
        source_val, target_val, Ws, Wt, ws_out, wt_out, w_int, bias)

    if "nc" not in _PROG_CACHE:
        _PROG_CACHE["nc"] = _build_program()
    nc = _PROG_CACHE["nc"]

    # host-side layout marshaling: d-major (transposed) views for the
    # projection matmuls, chunked by 128-partition groups
    tgtT_full = np.ascontiguousarray(
        target_val.transpose(0, 2, 1).reshape(B, N_DC, 128, T))
    in_maps = []
    for i in range(N_CORES):
        s_slice = source_val[:, i * S_LOC:(i + 1) * S_LOC, :]
        extra = {}
        salt = os.environ.get("ROUTE_BUILD_SALT", "")
        if salt:
            extra[f"salt_{salt}"] = np.zeros((128, 1), np.float32)
        in_maps.append({
            **extra,
            "srcT": np.ascontiguousarray(
                s_slice.transpose(0, 2, 1).reshape(B, N_DC, 128, S_LOC)),
            "tgtT": tgtT_full,
            "wsnT": wsnT,
            "wtnT": wtnT,
            "cols": cols,
            "wtoRep": wtoRep,
        })
    return nc, in_maps


def kernel(source_val, target_val, Ws, Wt, ws_out, wt_out, w_int, bias,
           _return_perf=None):
    from concourse.bass_utils import run_bass_kernel_spmd

    nc, in_maps = prepare(source_val, target_val, Ws, Wt, ws_out, wt_out,
                          w_int, bias)

    trace = bool(int(os.environ.get("ROUTE_TRACE", "0")))
    res = run_bass_kernel_spmd(nc, in_maps, core_ids=list(range(N_CORES)),
                               trace=trace)
    out = np.empty((B, S, T), np.float32)
    for i in range(N_CORES):
        out[:, i * S_LOC:(i + 1) * S_LOC, :] = res.results[i]["out"]
    if _return_perf is not None and isinstance(_return_perf, dict):
        _return_perf["exec_time_ns"] = res.exec_time_ns
        _return_perf["mean_exec_time_ns"] = res.mean_exec_time_ns
        _return_perf["trace"] = (res.instructions_and_trace or (None, None))[1]
    return out



# revision 12
# speedup vs baseline: 1.0927x; 1.0927x over previous
"""Trainium2 Bass kernel for nn_AdditiveLowRankRoute.

Math: out[b,s,t] = sum_w w_int[w]*silu(ps[b,s,w]*pt[b,t,w]) + s_lin[b,s] + t_lin[b,t] + bias
where ps = source_val @ Ws.T, pt = target_val @ Wt.T,
      s_lin = ps @ ws_out, t_lin = pt @ wt_out.

Key idea: silu(x) = x/2 + r(x) with r even. Fit per-w even polynomials
r(x) ~= sum_m c_{w,m} (x/X_w)^(2m) (coefficient-magnitude-constrained minimax
fits computed on host at call time from the actual data ranges). Then

  sum_w w_int*silu(ps*pt) = sum_w (w_int*ps/2)*pt                 <- 1 matmul
                          + sum_m sum_w [w_int*c_wm*an^2m]*[bn^2m] <- M matmuls

where an = ps/alpha_w, bn = pt/beta_w are computed on device via pre-scaled
projection weights. The whole interaction collapses into a K=(M+1)*128
fp32 matmul accumulated in PSUM; s_lin/t_lin/bias are fused into the PSUM
eviction. Work is sharded across 8 NeuronCores by the source row dim S.
"""
import os
import numpy as np

B, S, T, D, W = 2, 4096, 4096, 512, 128
N_CORES = 8
S_LOC = S // N_CORES          # 512 source rows per core (per batch)
M_POLY = 8                    # even powers 1..M_POLY
KMAX = 1000.0                 # L1 coefficient budget per w
MARG = 1.02                   # range margin
OCT = 512                     # t-tile width processed per inner block
N_OCT = T // OCT              # 8
N_SC = S_LOC // 128           # 4 source chunks of 128 rows
N_DC = D // 128               # 4 contraction chunks for projections


def _silu(x):
    return x / (1.0 + np.exp(-x))


def _fit_even_poly(X, M, kmax):
    """Minimax-ish fit of r(x)=silu(x)-x/2 by sum_m c_m (x/X)^(2m) on [-X, X]
    subject to sum|c_m| <= kmax. Returns c[M+1] (m=0..M)."""
    npts = 801
    u = np.cos(np.linspace(0, np.pi, npts))
    r = _silu(u * X) - u * X / 2
    V = np.stack([u ** (2 * m) for m in range(M + 1)], axis=1)
    try:
        from scipy.optimize import linprog

        n = M + 1
        A_ub = np.block([
            [V, -V, -np.ones((npts, 1))],
            [-V, V, -np.ones((npts, 1))],
            [np.ones((1, n)), np.ones((1, n)), np.zeros((1, 1))],
        ])
        b_ub = np.concatenate([r, -r, [kmax]])
        cvec = np.zeros(2 * n + 1)
        cvec[-1] = 1.0
        res = linprog(cvec, A_ub=A_ub, b_ub=b_ub,
                      bounds=[(0, None)] * (2 * n + 1), method="highs")
        if res.status == 0:
            return res.x[:n] - res.x[n:2 * n]
    except Exception:
        pass
    # numpy fallback: IRLS toward minimax + ridge scan for the kappa budget
    best = None
    for lam in np.logspace(-14, -2, 13):
        wts = np.ones(npts)
        c = None
        for _ in range(25):
            A = V * wts[:, None]
            G = A.T @ A + lam * np.eye(M + 1)
            c = np.linalg.solve(G, A.T @ (r * wts))
            res_ = np.abs(V @ c - r)
            wts = np.sqrt(wts * np.maximum(res_, 1e-12)
                          / np.maximum(res_.mean(), 1e-12))
            wts /= wts.mean()
        k = np.abs(c).sum()
        err = np.abs(V @ c - r).max()
        if k <= kmax and (best is None or err < best[1]):
            best = (c, err)
    assert best is not None
    return best[0]


# ----------------------------------------------------------------------------
# Device program
# ----------------------------------------------------------------------------
_PROG_CACHE = {}


def _build_program():
    import concourse.bacc as bacc
    import concourse.mybir as mybir
    import concourse.tile as tile

    fp32 = mybir.dt.float32
    AF = mybir.ActivationFunctionType
    ALU = mybir.AluOpType

    QT = 1024                  # t width per quarter (tgt load + out flush unit)
    N_Q = T // QT              # 4
    OPQ = QT // OCT            # octs per quarter: 2

    nc = bacc.Bacc(None, target_bir_lowering=False)
    reps = int(os.environ.get("ROUTE_REPS", "1"))
    mode = os.environ.get("ROUTE_MODE", "fp32")
    feat_dt = mybir.dt.float32r if mode == "f32r" else fp32
    salt = os.environ.get("ROUTE_BUILD_SALT", "")
    salt_d = None
    if salt:
        salt_d = nc.dram_tensor(f"salt_{salt}", (128, 1), fp32,
                                kind="ExternalInput")
    srcT_d = nc.dram_tensor("srcT", (B, N_DC, 128, S_LOC), fp32, kind="ExternalInput")
    tgtT_d = nc.dram_tensor("tgtT", (B, N_DC, 128, T), fp32, kind="ExternalInput")
    wsnT_d = nc.dram_tensor("wsnT", (N_DC, 128, W), fp32, kind="ExternalInput")
    wtnT_d = nc.dram_tensor("wtnT", (N_DC, 128, W), fp32, kind="ExternalInput")
    # per-partition (w) columns: 0=linA, 1=mpt, 2=wso_mv, 3..3+M-1=coefA(m=1..M),
    # 15=affine const (replicated)
    cols_d = nc.dram_tensor("cols", (W, 16), fp32, kind="ExternalInput")
    wtoR_d = nc.dram_tensor("wtoRep", (W, 128), fp32, kind="ExternalInput")
    out_d = nc.dram_tensor("out", (B, S_LOC, T), fp32, kind="ExternalOutput")

    with tile.TileContext(nc) as tc:
        with (
            tc.tile_pool(name="const", bufs=1) as cpool,
            tc.tile_pool(name="aside", bufs=1) as apool,
            tc.tile_pool(name="achain", bufs=2) as acpool,
            tc.tile_pool(name="bside", bufs=int(os.environ.get("ROUTE_BBUFS", "2")) ) as bpool,
            tc.tile_pool(name="tgtp", bufs=2) as tpool,
            tc.tile_pool(name="srcp", bufs=1) as spool,
            tc.tile_pool(name="stgp", bufs=1) as gpool,
            tc.tile_pool(name="ps_big", bufs=int(os.environ.get("ROUTE_PSBIG", "3")), space="PSUM") as ps_big,
            tc.tile_pool(name="ps_proj", bufs=2, space="PSUM") as ps_proj,
            tc.tile_pool(name="ps_tb", bufs=1, space="PSUM") as ps_tb,
            tc.tile_pool(name="ps_sl", bufs=1, space="PSUM") as ps_sl,
        ):
            wsnT = cpool.tile([128, N_DC, W], fp32, tag="wsnT")
            wtnT = cpool.tile([128, N_DC, W], fp32, tag="wtnT")
            cols = cpool.tile([W, 16], fp32, tag="cols")
            wtoR = cpool.tile([W, 128], fp32, tag="wtoR")
            for c in range(N_DC):
                nc.sync.dma_start(wsnT[:, c, :], wsnT_d[c])
                nc.sync.dma_start(wtnT[:, c, :], wtnT_d[c])
            nc.sync.dma_start(cols[:], cols_d[:])
            nc.sync.dma_start(wtoR[:], wtoR_d[:])
            if salt_d is not None:
                salt_t = cpool.tile([128, 1], fp32, tag="salt")
                nc.sync.dma_start(salt_t[:], salt_d[:])

            for _rep in range(reps):
                for b in range(B):
                    # ---- A side: an[w, s] for this b ----
                    srcT = spool.tile([128, N_DC, S_LOC], fp32, tag="srcT")
                    for c in range(N_DC):
                        nc.sync.dma_start(srcT[:, c, :], srcT_d[b, c])
                    pa_n = ps_proj.tile([128, S_LOC], fp32, tag="p_proj")
                    for c in range(N_DC):
                        nc.tensor.matmul(pa_n[:], wsnT[:, c, :], srcT[:, c, :],
                                         start=(c == 0), stop=(c == N_DC - 1))
                    an = apool.tile([W, S_LOC], fp32, tag="an")
                    nc.scalar.copy(an[:], pa_n[:])

                    # s_lin columns, one per source chunk: [128, 1] each
                    slin = apool.tile([W, N_SC], fp32, tag="slin")
                    for sc in range(N_SC):
                        p_sl = ps_sl.tile([128, 1], fp32, tag="p_sl")
                        nc.tensor.matmul(p_sl[:], an[:, sc * 128:(sc + 1) * 128],
                                         cols[:, 2:3], start=True, stop=True)
                        nc.scalar.copy(slin[:, sc:sc + 1], p_sl[:])

                    # A features: Af0 = linA*an ; Af[m] = coefA_m * (an^2)^m
                    a2 = apool.tile([W, S_LOC], fp32, tag="a2")
                    nc.vector.tensor_mul(a2[:], an[:], an[:])
                    afs = []
                    af0 = apool.tile([W, S_LOC], feat_dt, tag="af0")
                    nc.vector.tensor_scalar_mul(af0[:], an[:], cols[:, 0:1])
                    afs.append(af0)
                    pa_prev = a2
                    for m in range(1, M_POLY + 1):
                        if m > 1:
                            pa_m = acpool.tile([W, S_LOC], fp32, tag="pachain")
                            nc.vector.tensor_mul(pa_m[:], pa_prev[:], a2[:])
                            pa_prev = pa_m
                        af = apool.tile([W, S_LOC], feat_dt, tag=f"af{m}")
                        nc.vector.tensor_scalar_mul(af[:], pa_prev[:],
                                                    cols[:, 2 + m:3 + m])
                        afs.append(af)

                    # ---- B side + big matmul, per t quarter ----
                    for q in range(N_Q):
                        tq0 = q * QT
                        tgtT = tpool.tile([128, N_DC, QT], fp32, tag="tgtT")
                        for c in range(N_DC):
                            nc.sync.dma_start(tgtT[:, c, :],
                                              tgtT_d[b, c, :, tq0:tq0 + QT])
                        stgs = [gpool.tile([128, QT], fp32, tag=f"stg{sc}",
                                           name=f"stg{b}_{q}_{sc}")
                                for sc in range(N_SC)]
                        for o in range(OPQ):
                            t0 = o * OCT
                            p_bn = ps_proj.tile([128, OCT], fp32, tag="p_proj")
                            for c in range(N_DC):
                                nc.tensor.matmul(p_bn[:],
                                                 wtnT[:, c, :],
                                                 tgtT[:, c, t0:t0 + OCT],
                                                 start=(c == 0), stop=(c == N_DC - 1))
                            bn = bpool.tile([W, OCT], fp32, tag="bn")
                            nc.scalar.copy(bn[:], p_bn[:])

                            # tbase[j, t] = t_lin[t] (all rows equal) + (bias+const)
                            p_tb = ps_tb.tile([128, OCT], fp32, tag="p_tb")
                            nc.tensor.matmul(p_tb[:], wtoR[:], bn[:],
                                             start=True, stop=True)
                            tbase = bpool.tile([128, OCT], fp32, tag="tbase")
                            nc.scalar.activation(tbase[:], p_tb[:], AF.Identity,
                                                 bias=cols[:, 15:16])

                            blin = bpool.tile([W, OCT], feat_dt, tag="blin")
                            nc.vector.tensor_scalar_mul(blin[:], bn[:], cols[:, 1:2])
                            # square-tree: fp32 powers of b2 at {1,2,3,4,8} via
                            # ACT Square + DVE muls; features composed with a
                            # single rounding into feat_dt
                            p = {}
                            for mm_ in (1, 2, 4, 8):
                                p[mm_] = bpool.tile([W, OCT], fp32, tag=f"p{mm_}",
                                                    name=f"p{mm_}_{b}_{q}_{o}")
                            nc.scalar.square(p[1][:], bn[:])
                            nc.scalar.square(p[2][:], p[1][:])
                            nc.scalar.square(p[4][:], p[2][:])
                            nc.scalar.square(p[8][:], p[4][:])
                            p[3] = bpool.tile([W, OCT], fp32, tag="p3",
                                              name=f"p3_{b}_{q}_{o}")
                            nc.vector.tensor_mul(p[3][:], p[1][:], p[2][:])
                            comp = {5: (1, 4), 6: (2, 4), 7: (3, 4), 9: (1, 8),
                                    10: (2, 8), 11: (3, 8), 12: (4, 8)}
                            bfs = [blin]
                            for m in range(1, M_POLY + 1):
                                if m in p:
                                    if feat_dt is fp32:
                                        bf = p[m]
                                    else:
                                        bf = bpool.tile([W, OCT], feat_dt,
                                                        tag=f"bf{m}",
                                                        name=f"bf{m}_{b}_{q}_{o}")
                                        nc.vector.tensor_copy(bf[:], p[m][:])
                                else:
                                    i, j = comp[m]
                                    bf = bpool.tile([W, OCT], feat_dt,
                                                    tag=f"bf{m}",
                                                    name=f"bf{m}_{b}_{q}_{o}")
                                    nc.vector.tensor_mul(bf[:], p[i][:], p[j][:])
                                bfs.append(bf)

                            for sc in range(N_SC):
                                po = ps_big.tile([128, OCT], fp32, tag="po")
                                s_sl = slice(sc * 128, (sc + 1) * 128)
                                nc.tensor.matmul(po[:], afs[0][:, s_sl], blin[:],
                                                 start=True, stop=False)
                                for m in range(1, M_POLY + 1):
                                    nc.tensor.matmul(po[:], afs[m][:, s_sl],
                                                     bfs[m][:],
                                                     start=False, stop=(m == M_POLY))
                                nc.vector.scalar_tensor_tensor(
                                    stgs[sc][:, t0:t0 + OCT], po[:],
                                    slin[:, sc:sc + 1], tbase[:],
                                    op0=ALU.add, op1=ALU.add)
                        for sc in range(N_SC):
                            nc.scalar.dma_start(
                                out_d[b, sc * 128:(sc + 1) * 128, tq0:tq0 + QT],
                                stgs[sc][:])

    nc.compile()
    return nc


def _prep_constants(source_val, target_val, Ws, Wt, ws_out, wt_out, w_int, bias):
    """Host-side: data ranges, polynomial fits, packed constant tensors."""
    ps = np.einsum("bsd,wd->bsw", source_val, Ws).astype(np.float32)
    pt = np.einsum("btd,wd->btw", target_val, Wt).astype(np.float32)
    mps = np.abs(ps).max(axis=(0, 1)).astype(np.float64) * MARG
    mpt = np.abs(pt).max(axis=(0, 1)).astype(np.float64) * MARG
    mps = np.maximum(mps, 1e-6)
    mpt = np.maximum(mpt, 1e-6)
    Xw = mps * mpt

    CO = np.zeros((W, M_POLY + 1))
    for w in range(W):
        CO[w] = _fit_even_poly(Xw[w], M_POLY, KMAX)

    w_int64 = w_int.astype(np.float64)
    cols = np.zeros((W, 16), np.float64)
    cols[:, 0] = w_int64 * mps / 2.0                      # linA (scales an -> A_lin)
    cols[:, 1] = mpt                                      # bn -> pt
    cols[:, 2] = mps * ws_out.astype(np.float64)          # s_lin moving vector
    for m in range(1, M_POLY + 1):
        cols[:, 2 + m] = w_int64 * CO[:, m]               # coefA m=1..M
    const_term = float((w_int64 * CO[:, 0]).sum() + float(bias))
    cols[:, 15] = const_term
    wtoRep = np.repeat((mpt * wt_out.astype(np.float64))[:, None], 128, axis=1)

    wsnT = np.ascontiguousarray(
        (Ws.astype(np.float64) / mps[:, None]).T.reshape(N_DC, 128, W))
    wtnT = np.ascontiguousarray(
        (Wt.astype(np.float64) / mpt[:, None]).T.reshape(N_DC, 128, W))
    return (cols.astype(np.float32), wtoRep.astype(np.float32),
            wsnT.astype(np.float32), wtnT.astype(np.float32))


def prepare(source_val, target_val, Ws, Wt, ws_out, wt_out, w_int, bias):
    source_val = np.ascontiguousarray(np.asarray(source_val, np.float32))
    target_val = np.ascontiguousarray(np.asarray(target_val, np.float32))
    Ws = np.asarray(Ws, np.float32)
    Wt = np.asarray(Wt, np.float32)
    ws_out = np.asarray(ws_out, np.float32)
    wt_out = np.asarray(wt_out, np.float32)
    w_int = np.asarray(w_int, np.float32)

    cols, wtoRep, wsnT, wtnT = _prep_constants(
        source_val, target_val, Ws, Wt, ws_out, wt_out, w_int, bias)

    if "nc" not in _PROG_CACHE:
        _PROG_CACHE["nc"] = _build_program()
    nc = _PROG_CACHE["nc"]

    # host-side layout marshaling: d-major (transposed) views for the
    # projection matmuls, chunked by 128-partition groups
    tgtT_full = np.ascontiguousarray(
        target_val.transpose(0, 2, 1).reshape(B, N_DC, 128, T))
    in_maps = []
    for i in range(N_CORES):
        s_slice = source_val[:, i * S_LOC:(i + 1) * S_LOC, :]
        extra = {}
        salt = os.environ.get("ROUTE_BUILD_SALT", "")
        if salt:
            extra[f"salt_{salt}"] = np.zeros((128, 1), np.float32)
        in_maps.append({
            **extra,
            "srcT": np.ascontiguousarray(
                s_slice.transpose(0, 2, 1).reshape(B, N_DC, 128, S_LOC)),
            "tgtT": tgtT_full,
            "wsnT": wsnT,
            "wtnT": wtnT,
            "cols": cols,
            "wtoRep": wtoRep,
        })
    return nc, in_maps


def kernel(source_val, target_val, Ws, Wt, ws_out, wt_out, w_int, bias,
           _return_perf=None):
    from concourse.bass_utils import run_bass_kernel_spmd

    nc, in_maps = prepare(source_val, target_val, Ws, Wt, ws_out, wt_out,
                          w_int, bias)

    trace = bool(int(os.environ.get("ROUTE_TRACE", "0")))
    res = run_bass_kernel_spmd(nc, in_maps, core_ids=list(range(N_CORES)),
                               trace=trace)
    out = np.empty((B, S, T), np.float32)
    for i in range(N_CORES):
        out[:, i * S_LOC:(i + 1) * S_LOC, :] = res.results[i]["out"]
    if _return_perf is not None and isinstance(_return_perf, dict):
        _return_perf["exec_time_ns"] = res.exec_time_ns
        _return_perf["mean_exec_time_ns"] = res.mean_exec_time_ns
        _return_perf["trace"] = (res.instructions_and_trace or (None, None))[1]
    return out

